# revision 51
# baseline (speedup 1.0000x reference)
"""Trainium2 Bass kernel for nn_DecSwitchedDeconv — PE-array-tiled per-sample convs.

Strategy (data-parallel, 32 samples/core, groups of 8):
  - conv1 runs as 8 concurrent (64x32) PE tiles (tile_position row/col groups),
    conv2 as 8 concurrent (32x64) tiles: one sample per tile, zero wasted MACs.
  - Dense-output chunking: each conv is 2 chunks of 16 output rows; the matmul
    rhs is a 3D window AP ([64, 16 rows @ stride 34, 32 cols]) over the padded
    input, so PSUM/outputs are dense 512-wide slabs.
  - All routing/gather on host: per-sample weights gathered by y_index,
    kernels flipped/transposed, z folded into W2, b2*z folded into the
    residual input xq = x + b2*z (dense [*,1024] bf16).
  - v2 schedule: all input DMAs issued up-front (io/wp pools hold all 4
    groups), xq(g0) split across sync+gpsimd queues in quadrant chunks so
    relu(g0) fires as chunks land; weights + biases on the scalar queue;
    ACT-table warmed by a dummy activation at the head; ~16 N=512 warmup
    matmuls cover the PE until conv1(g0) is ready (~5.5us) and keep HAM
    warm.  Steady state: conv1 evacs (bias+relu) both on scalar ACT,
    relu(g1..g3) on gpsimd (+vector for g1), conv2 epilogue (psum + xq)
    on vector only (only DVE can read PSUM and tensor-tensor), stores all
    on the sync queue as [128,1024] per output row tile.  Pair-structured
    PE emission conv1(2p), conv1(2p+1), conv2(2p), conv2(2p+1) halves PE
    tiling-mode switches; one shared PSUM pool (bufs=8) rotates all banks.
"""

import numpy as np

import concourse.bacc as bacc
import concourse.bass as bass
import concourse.mybir as mybir
import concourse.tile as tile
from concourse.bass_utils import run_bass_kernel_spmd

B, C, CSM, NB, HW = 256, 64, 32, 8, 32
M = 8                    # cores
BS = B // M              # 32 samples/core
NG = BS // 8             # 4 groups of 8 samples
WP = HW + 2              # 34
L = WP * WP              # 1156

f32 = mybir.dt.float32
bf16 = mybir.dt.bfloat16
fp8 = mybir.dt.float8e4


def _build_bass():
    nc = bacc.Bacc(target_bir_lowering=False, debug=False)
    # per-core inputs (host pre-gathered/packed, all static)
    # xqg is quadrant-major [(g,k,pair),1024] so every [128,1024] chunk load
    # is a contiguous 256KB HBM read (strided reads gate the ramp otherwise)
    xqg = nc.dram_tensor("xqg", [NG * 4 * 128, 1024], bf16, kind="ExternalInput")
    # wg split into contiguous conv1/conv2 halves: block (g,0)=w1, (g,1)=w2
    wg = nc.dram_tensor("wg", [NG * 2 * 128, 1152], fp8, kind="ExternalInput")
    b1g = nc.dram_tensor("b1g", [128, 2 * NG], f32, kind="ExternalInput")
    b2zg = nc.dram_tensor("b2zg", [128, 4 * NG], f32, kind="ExternalInput")
    outd = nc.dram_tensor("out", [NG * 4 * 128, 1024], bf16, kind="ExternalOutput")

    add = mybir.AluOpType.add
    amax = mybir.AluOpType.max
    Relu = mybir.ActivationFunctionType.Relu

    with tile.TileContext(nc) as tc:
        # persistent padded tensors; borders zeroed once and never rewritten
        xpads, hpas, hpbs = [], [], []
        for i in range(2):
            xp = nc.alloc_sbuf_tensor(f"xpad{i}", [128, 4 * L], bf16).ap()
            ha = nc.alloc_sbuf_tensor(f"hpa{i}", [128, L], bf16).ap()
            hb = nc.alloc_sbuf_tensor(f"hpb{i}", [128, L], bf16).ap()
            xpads.append(xp)
            hpas.append(ha)
            hpbs.append(hb)
        warm_sb = nc.alloc_sbuf_tensor("warm_sb", [128, 1152], bf16).ap()
        b1t = nc.alloc_sbuf_tensor("b1t", [128, 2 * NG], f32).ap()
        b2zt = nc.alloc_sbuf_tensor("b2zt", [128, 4 * NG], f32).ap()

        def zero_borders(eng, t):
            # pad frame of one 34x34 block: row 0, row 33, cols 0/33
            eng.memset(t[:, 0:WP], 0.0)
            eng.memset(t[:, 33 * WP:L], 0.0)
            sides = t[:, 33:33 + 33 * WP].rearrange(
                "p (r w) -> p r w", w=WP)[:, :, 0:2]
            eng.memset(sides, 0.0)

        act_scr = nc.alloc_sbuf_tensor("act_scr", [128, 2], f32).ap()

        with (
            tc.tile_pool(name="io", bufs=4) as iop,
            tc.tile_pool(name="wp", bufs=4) as wpp,
            tc.tile_pool(name="osp", bufs=3) as ospp,
            tc.tile_pool(name="ps", bufs=8, space="PSUM") as psp,
        ):
            # ---- head: queue all input DMAs immediately ----
            xq_tiles, w_tiles = [], []
            for g in range(NG):
                xq_tiles.append(iop.tile([128, 4 * 1024], bf16, tag="xq",
                                         name=f"xq_{g}"))
                w_tiles.append(wpp.tile([128, 2304], fp8, tag="w",
                                        name=f"w_{g}"))
            # all xq loads are contiguous per-quadrant [128,1024] chunks.
            # g0 split sync/gpsimd for earliest landing; later groups queue
            # behind on sync - its ~0.7us/instr issue rate self-paces them.
            def load_xq(g, k, eng):
                r0 = (g * 4 + k) * 128
                eng.dma_start(xq_tiles[g][:, k * 1024:(k + 1) * 1024],
                              xqg.ap()[r0:r0 + 128, :])

            def load_w(g, half, eng):
                r0 = (g * 2 + half) * 128
                eng.dma_start(w_tiles[g][:, half * 1152:(half + 1) * 1152],
                              wg.ap()[r0:r0 + 128, :])

            # half-quadrant loads for g0 so relu can chase the DMA closely
            def load_xq_half(g, k, h, eng):
                r0 = (g * 4 + k) * 128
                eng.dma_start(
                    xq_tiles[g][:, k * 1024 + h * 512:k * 1024 + (h + 1) * 512],
                    xqg.ap()[r0:r0 + 128, h * 512:(h + 1) * 512])

            # wave 1 (critical for conv1(g0)): xq(g0) + w1(g0) + biases,
            # balanced across the three queues so they finish together
            load_w(0, 0, nc.gpsimd)                 # w1(g0), 148KB
            load_xq_half(0, 0, 0, nc.sync)
            nc.scalar.dma_start(b2zt, b2zg.ap())
            nc.scalar.dma_start(b1t, b1g.ap())
            load_xq_half(0, 2, 0, nc.scalar)
            load_xq_half(0, 0, 1, nc.sync)
            load_xq_half(0, 2, 1, nc.scalar)
            load_xq_half(0, 3, 0, nc.gpsimd)
            load_xq_half(0, 1, 0, nc.sync)
            load_xq_half(0, 1, 1, nc.scalar)
            load_xq_half(0, 3, 1, nc.gpsimd)
            # wave 2: g1's inputs
            load_xq(1, 0, nc.sync)
            load_xq(1, 2, nc.gpsimd)
            load_w(1, 0, nc.scalar)                 # w1(g1)
            load_xq(1, 1, nc.sync)
            load_xq(1, 3, nc.gpsimd)
            # wave 3: conv2 weights + later groups
            load_w(0, 1, nc.gpsimd)                 # w2(g0)
            load_w(1, 1, nc.scalar)                 # w2(g1)
            for g in range(2, NG):
                for k in range(4):
                    load_xq(g, k, nc.sync)

            # border memsets: vector does xpad0, gpsimd does hpa0/hpb0
            xp0q = xpads[0].rearrange("p (b l) -> p b l", b=4)
            xp1q = xpads[1].rearrange("p (b l) -> p b l", b=4)
            for k in range(4):
                zero_borders(nc.vector, xp0q[:, k])
            zero_borders(nc.gpsimd, hpas[0])
            zero_borders(nc.gpsimd, hpbs[0])

            # ACT table warm on a private scratch (no deps with real tiles)
            nc.scalar.activation(act_scr[:, 1:2], act_scr[:, 0:1], Relu)

            # HAM pre-warm: dummy full-array matmuls while first loads land
            wps = psp.tile([128, 512], f32, name="warm_ps", tag="ps")
            for i in range(22):
                nc.tensor.matmul(
                    wps[:, :], lhsT=warm_sb[:, 1024:1152],
                    rhs=warm_sb[:, 0:512], start=True, stop=True)

            def emit_relu(g, quads):
                # quads: list of (k, engine) quadrant assignments.
                # vector: tensor_scalar add+max; scalar: ACT relu w/ bias.
                # (gpsimd tensor_scalar is ~30x slower on this shape - avoid.)
                xqall = xq_tiles[g]
                xp3 = xpads[g % 2].rearrange("p (b h w) -> p b h w", b=4, w=WP)
                for k, eng in quads:
                    dst = xp3[:, k, 1:HW + 1, 1:HW + 1]
                    src = xqall[:, k * 1024:(k + 1) * 1024].rearrange(
                        "p (h w) -> p h w", w=HW)
                    if eng is nc.scalar:
                        eng.activation(dst, src, Relu,
                                       bias=b2zt[:, 4 * g + k:4 * g + k + 1])
                    else:
                        eng.tensor_scalar(
                            dst, src, b2zt[:, 4 * g + k:4 * g + k + 1], 0.0,
                            op0=add, op1=amax)

            def emit_conv1(g):
                w1sb = w_tiles[g][:, 0:1152]
                xp4 = xpads[g % 2].rearrange("p (b h w) -> p b h w", b=4, w=WP)
                ha3 = hpas[g % 2].rearrange("p (h w) -> p h w", w=WP)
                hb3 = hpbs[g % 2].rearrange("p (h w) -> p h w", w=WP)
                for c in range(2):
                    r0 = 16 * c
                    psa = psp.tile([128, 512], f32, name=f"ps1a_{g}_{c}", tag="ps")
                    psb = psp.tile([128, 512], f32, name=f"ps1b_{g}_{c}", tag="ps")
                    for t in range(9):
                        dy, dx = divmod(t, 3)
                        for k in range(8):
                            row = 64 * (k // 4)
                            col = 32 * (k % 4)
                            ps = psa if k < 4 else psb
                            nc.tensor.matmul(
                                ps[col:col + 32, :],
                                lhsT=w1sb[row:row + 64,
                                          (k % 4) * 288 + t * 32:(k % 4) * 288 + (t + 1) * 32],
                                rhs=xp4[row:row + 64, k % 4,
                                        r0 + dy:r0 + dy + 16, dx:dx + HW],
                                start=(t == 0), stop=(t == 8),
                                tile_position=(row, col),
                            )
                    # evacs on scalar; the last chunk's bank-b goes to
                    # vector (idle then) so all banks free ~0.7us sooner -
                    # the next conv2's psum allocation waits on these
                    nc.scalar.activation(
                        ha3[:, r0 + 1:r0 + 17, 1:HW + 1],
                        psa.rearrange("p (h w) -> p h w", w=HW),
                        Relu, bias=b1t[:, 2 * g:2 * g + 1])
                    if c == 1:
                        nc.vector.tensor_scalar(
                            hb3[:, r0 + 1:r0 + 17, 1:HW + 1],
                            psb.rearrange("p (h w) -> p h w", w=HW),
                            b1t[:, 2 * g + 1:2 * g + 2], 0.0,
                            op0=add, op1=amax)
                    else:
                        nc.scalar.activation(
                            hb3[:, r0 + 1:r0 + 17, 1:HW + 1],
                            psb.rearrange("p (h w) -> p h w", w=HW),
                            Relu, bias=b1t[:, 2 * g + 1:2 * g + 2])

            def emit_conv2(g):
                w2sb = w_tiles[g][:, 1152:2304]
                xqall = xq_tiles[g]
                ha3 = hpas[g % 2].rearrange("p (h w) -> p h w", w=WP)
                hb3 = hpbs[g % 2].rearrange("p (h w) -> p h w", w=WP)
                outs = [ospp.tile([128, 1024], bf16, tag=f"os{r}",
                                  name=f"os{r}_{g}") for r in range(4)]
                chunks = ((0, 16), (16, 16))
                for ci, (r0, nr) in enumerate(chunks):
                    nw = nr * HW
                    pss = [psp.tile([128, nw], f32, name=f"ps2_{g}_{ci}_{r}",
                                    tag="ps")
                           for r in range(4)]
                    for t in range(9):
                        dy, dx = divmod(t, 3)
                        for k in range(8):
                            r, h = divmod(k, 2)
                            src3 = ha3 if h == 0 else hb3
                            nc.tensor.matmul(
                                pss[r][64 * h:64 * h + 64, :],
                                lhsT=w2sb[32 * r:32 * r + 32,
                                          h * 576 + t * 64:h * 576 + (t + 1) * 64],
                                rhs=src3[32 * r:32 * r + 32,
                                         r0 + dy:r0 + dy + nr, dx:dx + HW],
                                start=(t == 0), stop=(t == 8),
                                tile_position=(32 * r, 64 * h),
                            )
                    o0 = r0 * HW
                    for r in range(4):
                        nc.vector.tensor_tensor(
                            outs[r][:, o0:o0 + nw], pss[r][:, :],
                            xqall[:, r * 1024 + o0:r * 1024 + o0 + nw],
                            op=add)
                        row0 = (g * 4 + r) * 128
                        if g >= NG - 2:
                            # last two groups: store per chunk, spread over
                            # all three queues, so almost no transfer remains
                            # after the final epilogue op
                            eng = (nc.sync, nc.sync, nc.gpsimd, nc.scalar)[r]
                            eng.dma_start(
                                outd.ap()[row0:row0 + 128, o0:o0 + nw],
                                outs[r][:, o0:o0 + nw])
                        elif ci == 1:
                            eng = nc.sync if r < 2 else nc.gpsimd
                            eng.dma_start(outd.ap()[row0:row0 + 128, :],
                                          outs[r][:, :])

            # relu(g0): half-quadrant ops on vector, in DMA-landing order
            xp3g0 = xpads[0].rearrange("p (b h w) -> p b h w", b=4, w=WP)
            for k, h in ((0, 0), (2, 0), (0, 1), (2, 1),
                         (1, 0), (3, 0), (1, 1), (3, 1)):
                dst = xp3g0[:, k, 1 + 16 * h:17 + 16 * h, 1:HW + 1]
                src = xq_tiles[0][:, k * 1024 + h * 512:
                                  k * 1024 + (h + 1) * 512].rearrange(
                    "p (h w) -> p h w", w=HW)
                nc.vector.tensor_scalar(
                    dst, src, b2zt[:, k:k + 1], 0.0, op0=add, op1=amax)

            # remaining border memsets (needed from conv1(g1)/conv2(g1) on)
            for k in range(4):
                zero_borders(nc.vector, xp1q[:, k])
            zero_borders(nc.gpsimd, hpas[1])
            zero_borders(nc.gpsimd, hpbs[1])

            # ---- pipeline: pair structure ----
            # relu(g1) all vector: keeps scalar free for evac(g0)
            emit_relu(1, [(0, nc.vector), (1, nc.vector),
                          (2, nc.vector), (3, nc.vector)])
            emit_conv1(0)
            # late weight loads issued from scalar after evac(g0) (~12us):
            # keeps the head DMA window clear for xq(g0/g1)+wg(g0/g1)
            for g in (2, 3):
                for half in (0, 1):
                    load_w(g, half, nc.scalar)
            emit_relu(2, [(0, nc.vector), (1, nc.vector),
                          (2, nc.vector), (3, nc.vector)])
            emit_conv1(1)
            emit_conv2(0)
            emit_relu(3, [(0, nc.vector), (1, nc.vector),
                          (2, nc.scalar), (3, nc.scalar)])
            emit_conv2(1)
            emit_conv1(2)
            emit_conv1(3)
            emit_conv2(2)
            emit_conv2(3)

    nc.compile()
    return nc


import os as _os
if _os.environ.get("LDWOPT", "0") == "1":
    import concourse.bass_utils as _bu
    if not getattr(_bu, "_ldw_patched", False):
        _orig = _bu.run_command
        def _rc(argv, **kw):
            argv = ["--enable-ldw-opt=true" if a == "--enable-ldw-opt=false" else a
                    for a in argv]
            return _orig(argv, **kw)
        _bu.run_command = _rc
        _bu._ldw_patched = True

_NC = None


def _get_nc():
    global _NC
    if _NC is None:
        _NC = _build_bass()
    return _NC


def _host_prep(x, y_index, z, W1, b1, W2, b2):
    import ml_dtypes
    idx = np.asarray(y_index).reshape(B).astype(np.int64)
    # flipped-kernel stacks: w1t [NB, C, 9, CSM], w2t [NB, CSM, 9, C]
    w1t = np.ascontiguousarray(
        W1[:, :, :, ::-1, ::-1].transpose(0, 1, 3, 4, 2)).reshape(NB, C, 9, CSM)
    w2t = np.ascontiguousarray(
        W2[:, :, :, ::-1, ::-1].transpose(0, 1, 3, 4, 2)).reshape(NB, CSM, 9, C)
    w1s = w1t[idx]                                   # [B, 64, 9, 32] f32
    w2s = w2t[idx] * z[:, None, None, :]             # [B, 32, 9, 64] f32
    b2z = b2[idx] * z                                # [B, 64]
    b1s = b1[idx]                                    # [B, 32]

    # xq = x + b2z, dense [B, 64, 1024]
    xq = (x + b2z[:, :, None, None]).reshape(B, C, HW * HW)
    xq_span = xq.astype(ml_dtypes.bfloat16)

    w1sb = w1s.astype(ml_dtypes.float8_e4m3fn)
    w2sb = w2s.astype(ml_dtypes.float8_e4m3fn)

    in_maps = []
    for cr in range(M):
        s0 = cr * BS
        # xqg rows: (g, k) pair tile = samples (s0+8g+k | s0+8g+4+k)
        xqg = np.empty((NG * 4 * 128, 1024), ml_dtypes.bfloat16)
        wgh = np.zeros((NG * 2 * 128, 1152), ml_dtypes.float8_e4m3fn)
        b1h = np.zeros((128, 2 * NG), np.float32)
        b2zh = np.zeros((128, 4 * NG), np.float32)
        for g in range(NG):
            for k in range(4):
                sa, sb = s0 + 8 * g + k, s0 + 8 * g + 4 + k
                q0 = (g * 4 + k) * 128
                xqg[q0:q0 + 64, :] = xq_span[sa]
                xqg[q0 + 64:q0 + 128, :] = xq_span[sb]
                # relu(x) = max(xq - b2z, 0): bias is ADDED by ACT/DVE
                b2zh[0:64, 4 * g + k] = -b2z[sa]
                b2zh[64:128, 4 * g + k] = -b2z[sb]
                # conv1 weights: tile k (cols k*288) top=sa, tile 8+k bottom=sb
                w1r0 = 2 * g * 128
                wgh[w1r0:w1r0 + 64,
                    k * 288:(k + 1) * 288] = w1sb[sa].reshape(64, 288)
                wgh[w1r0 + 64:w1r0 + 128,
                    k * 288:(k + 1) * 288] = w1sb[sb].reshape(64, 288)
                # conv1 bias: bank a (cols 2g) = samples sa at 32*k..; bank b = sb
                b1h[32 * k:32 * (k + 1), 2 * g] = b1s[sa]
                b1h[32 * k:32 * (k + 1), 2 * g + 1] = b1s[sb]
                # conv2 weights: tile (32k, 64h): h=0 -> sa, h=1 -> sb
                w2r0 = (2 * g + 1) * 128
                wgh[w2r0 + 32 * k:w2r0 + 32 * (k + 1),
                    0:576] = w2sb[sa].reshape(32, 576)
                wgh[w2r0 + 32 * k:w2r0 + 32 * (k + 1),
                    576:1152] = w2sb[sb].reshape(32, 576)
        in_maps.append(dict(xqg=xqg, wg=wgh, b1g=b1h, b2zg=b2zh))
    return in_maps


def kernel(x, y_index, y_hard, z, W1, b1, W2, b2, _trace=False):
    x = np.asarray(x, dtype=np.float32)
    z = np.asarray(z, dtype=np.float32)
    W1 = np.asarray(W1, dtype=np.float32)
    b1 = np.asarray(b1, dtype=np.float32)
    W2 = np.asarray(W2, dtype=np.float32)
    b2 = np.asarray(b2, dtype=np.float32)

    nc = _get_nc()
    in_maps = _host_prep(x, y_index, z, W1, b1, W2, b2)
    res = run_bass_kernel_spmd(nc, in_maps, core_ids=list(range(M)), trace=_trace)
    out = np.empty((B, C, HW, HW), np.float32)
    for cr in range(M):
        o = np.asarray(res.results[cr]["out"], dtype=np.float32)
        o = o.reshape(NG, 4, 2, C, HW, HW)
        for g in range(NG):
            for k in range(4):
                out[cr * BS + 8 * g + k] = o[g, k, 0]
                out[cr * BS + 8 * g + 4 + k] = o[g, k, 1]
    if _trace:
        kernel._last_results = res
    return out


# revision 52
# speedup vs baseline: 1.0339x; 1.0339x over previous
"""Trainium2 Bass kernel for nn_DecSwitchedDeconv — PE-array-tiled per-sample convs.

Strategy (data-parallel, 32 samples/core, 4 groups of 8):
  - conv1 runs as 8 concurrent (64x32) PE tiles (tile_position row/col groups),
    conv2 as 8 concurrent (32x64) tiles: one sample per tile, zero wasted MACs.
    Each conv is 2 chunks of 16 output rows; the matmul rhs is a 3D window AP
    over the padded input so PSUM/outputs are dense 512-wide slabs. Sustained
    round cadence ~285ns (512-col stream + LDW/dispatch overhead; walrus emits
    one LDWEIGHTS per matmul, so the weight-load port is a co-bottleneck).
  - All routing/gather on host: per-sample weights gathered by y_index, flipped
    + transposed, z folded into W2, b2*z folded into the residual xq = x+b2*z.
    Weights are fp8e4 (halves weight DMA; error well within tolerance), xq and
    the output stay bf16. DRAM layouts are quadrant-major so every DMA chunk
    is a contiguous 128-row read (strided reads crater the DMA rate).
  - Head: loads issued in priority waves over the three DMA-capable queues
    (sync/gpsimd/scalar, ~100GB/s each): wave 1 = xq(g0) half-quadrants +
    w1(g0) + biases balanced so they finish together; then g1, then the rest.
    relu(g0) chases the landings as 8 half-quadrant vector ops; 22 full-array
    warmup matmuls keep the PE busy (and the HAM clock-gate warm) until the
    first conv data lands (~DMA-bound at ~12us).
  - Steady state: pair-structured PE emission c1(2p), c1(2p+1), c2(2p),
    c2(2p+1) halves tiling-mode switches and hides the conv1->conv2 evac
    latency. conv1 evacs (bias+relu) on scalar ACT, except each group's last
    chunk bank-b on vector so all PSUM banks free early (the next conv2's
    bank rotation waits on them). relu(g1..g3) on vector/scalar only (gpsimd
    tensor_scalar is ~30x slower on this shape - never use it). conv2
    epilogue (psum + xq) on vector only (sole engine that can tensor-tensor
    from PSUM). Stores: [128,1024] per row tile on sync/gpsimd; the last two
    groups store per 512-chunk spread over all three queues so almost no DMA
    drain remains after the final epilogue op.
"""

import numpy as np

import concourse.bacc as bacc
import concourse.bass as bass
import concourse.mybir as mybir
import concourse.tile as tile
from concourse.bass_utils import run_bass_kernel_spmd

B, C, CSM, NB, HW = 256, 64, 32, 8, 32
M = 8                    # cores
BS = B // M              # 32 samples/core
NG = BS // 8             # 4 groups of 8 samples
WP = HW + 2              # 34
L = WP * WP              # 1156

f32 = mybir.dt.float32
bf16 = mybir.dt.bfloat16
fp8 = mybir.dt.float8e4


def _build_bass():
    nc = bacc.Bacc(target_bir_lowering=False, debug=False)
    # per-core inputs (host pre-gathered/packed, all static)
    # xqg is quadrant-major [(g,k,pair),1024] so every [128,1024] chunk load
    # is a contiguous 256KB HBM read (strided reads gate the ramp otherwise)
    xqg = nc.dram_tensor("xqg", [NG * 4 * 128, 1024], bf16, kind="ExternalInput")
    # wg split into contiguous conv1/conv2 halves: block (g,0)=w1, (g,1)=w2
    wg = nc.dram_tensor("wg", [NG * 2 * 128, 1152], fp8, kind="ExternalInput")
    b1g = nc.dram_tensor("b1g", [128, 2 * NG], f32, kind="ExternalInput")
    b2zg = nc.dram_tensor("b2zg", [128, 4 * NG], f32, kind="ExternalInput")
    outd = nc.dram_tensor("out", [NG * 4 * 128, 1024], bf16, kind="ExternalOutput")

    add = mybir.AluOpType.add
    amax = mybir.AluOpType.max
    Relu = mybir.ActivationFunctionType.Relu

    with tile.TileContext(nc) as tc:
        # persistent padded tensors; borders zeroed once and never rewritten
        xpads, hpas, hpbs = [], [], []
        for i in range(2):
            xp = nc.alloc_sbuf_tensor(f"xpad{i}", [128, 4 * L], bf16).ap()
            ha = nc.alloc_sbuf_tensor(f"hpa{i}", [128, L], bf16).ap()
            hb = nc.alloc_sbuf_tensor(f"hpb{i}", [128, L], bf16).ap()
            xpads.append(xp)
            hpas.append(ha)
            hpbs.append(hb)
        warm_sb = nc.alloc_sbuf_tensor("warm_sb", [128, 1152], bf16).ap()
        b1t = nc.alloc_sbuf_tensor("b1t", [128, 2 * NG], f32).ap()
        b2zt = nc.alloc_sbuf_tensor("b2zt", [128, 4 * NG], f32).ap()

        def zero_borders(eng, t):
            # pad frame of one 34x34 block: row 0, row 33, cols 0/33
            eng.memset(t[:, 0:WP], 0.0)
            eng.memset(t[:, 33 * WP:L], 0.0)
            sides = t[:, 33:33 + 33 * WP].rearrange(
                "p (r w) -> p r w", w=WP)[:, :, 0:2]
            eng.memset(sides, 0.0)

        act_scr = nc.alloc_sbuf_tensor("act_scr", [128, 2], f32).ap()

        with (
            tc.tile_pool(name="io", bufs=4) as iop,
            tc.tile_pool(name="wp", bufs=4) as wpp,
            tc.tile_pool(name="osp", bufs=3) as ospp,
            tc.tile_pool(name="ps", bufs=8, space="PSUM") as psp,
        ):
            # ---- head: queue all input DMAs immediately ----
            xq_tiles, w_tiles = [], []
            for g in range(NG):
                xq_tiles.append(iop.tile([128, 4 * 1024], bf16, tag="xq",
                                         name=f"xq_{g}"))
                w_tiles.append(wpp.tile([128, 2304], fp8, tag="w",
                                        name=f"w_{g}"))
            # all xq loads are contiguous per-quadrant [128,1024] chunks.
            # g0 split sync/gpsimd for earliest landing; later groups queue
            # behind on sync - its ~0.7us/instr issue rate self-paces them.
            def load_xq(g, k, eng):
                r0 = (g * 4 + k) * 128
                eng.dma_start(xq_tiles[g][:, k * 1024:(k + 1) * 1024],
                              xqg.ap()[r0:r0 + 128, :])

            def load_w(g, half, eng):
                r0 = (g * 2 + half) * 128
                eng.dma_start(w_tiles[g][:, half * 1152:(half + 1) * 1152],
                              wg.ap()[r0:r0 + 128, :])

            # half-quadrant loads for g0 so relu can chase the DMA closely
            def load_xq_half(g, k, h, eng):
                r0 = (g * 4 + k) * 128
                eng.dma_start(
                    xq_tiles[g][:, k * 1024 + h * 512:k * 1024 + (h + 1) * 512],
                    xqg.ap()[r0:r0 + 128, h * 512:(h + 1) * 512])

            # wave 1 (critical for conv1(g0)): xq(g0) + w1(g0) + biases,
            # balanced across the three queues so they finish together
            load_w(0, 0, nc.gpsimd)                 # w1(g0), 148KB
            load_xq_half(0, 0, 0, nc.sync)
            nc.scalar.dma_start(b2zt, b2zg.ap())
            nc.scalar.dma_start(b1t, b1g.ap())
            load_xq_half(0, 2, 0, nc.scalar)
            load_xq_half(0, 0, 1, nc.sync)
            load_xq_half(0, 2, 1, nc.scalar)
            load_xq_half(0, 3, 0, nc.gpsimd)
            load_xq_half(0, 1, 0, nc.sync)
            load_xq_half(0, 1, 1, nc.scalar)
            load_xq_half(0, 3, 1, nc.gpsimd)
            # wave 2: g1's inputs
            load_xq(1, 0, nc.sync)
            load_xq(1, 2, nc.gpsimd)
            load_w(1, 0, nc.scalar)                 # w1(g1)
            load_xq(1, 1, nc.sync)
            load_xq(1, 3, nc.gpsimd)
            # wave 3: conv2 weights + later groups
            load_w(0, 1, nc.gpsimd)                 # w2(g0)
            load_w(1, 1, nc.scalar)                 # w2(g1)
            for g in range(2, NG):
                for k in range(4):
                    load_xq(g, k, nc.sync)

            # border memsets: vector does xpad0, gpsimd does hpa0/hpb0
            xp0q = xpads[0].rearrange("p (b l) -> p b l", b=4)
            xp1q = xpads[1].rearrange("p (b l) -> p b l", b=4)
            for k in range(4):
                zero_borders(nc.vector, xp0q[:, k])
            zero_borders(nc.gpsimd, hpas[0])
            zero_borders(nc.gpsimd, hpbs[0])

            # ACT table warm on a private scratch (no deps with real tiles)
            nc.scalar.activation(act_scr[:, 1:2], act_scr[:, 0:1], Relu)

            # HAM pre-warm: dummy full-array matmuls while first loads land
            wps = psp.tile([128, 512], f32, name="warm_ps", tag="ps")
            for i in range(22):
                nc.tensor.matmul(
                    wps[:, :], lhsT=warm_sb[:, 1024:1152],
                    rhs=warm_sb[:, 0:512], start=True, stop=True)

            def emit_relu(g, quads):
                # quads: list of (k, engine) quadrant assignments.
                # vector: tensor_scalar add+max; scalar: ACT relu w/ bias.
                # (gpsimd tensor_scalar is ~30x slower on this shape - avoid.)
                xqall = xq_tiles[g]
                xp3 = xpads[g % 2].rearrange("p (b h w) -> p b h w", b=4, w=WP)
                for k, eng in quads:
                    dst = xp3[:, k, 1:HW + 1, 1:HW + 1]
                    src = xqall[:, k * 1024:(k + 1) * 1024].rearrange(
                        "p (h w) -> p h w", w=HW)
                    if eng is nc.scalar:
                        eng.activation(dst, src, Relu,
                                       bias=b2zt[:, 4 * g + k:4 * g + k + 1])
                    else:
                        eng.tensor_scalar(
                            dst, src, b2zt[:, 4 * g + k:4 * g + k + 1], 0.0,
                            op0=add, op1=amax)

            def emit_conv1(g):
                w1sb = w_tiles[g][:, 0:1152]
                xp4 = xpads[g % 2].rearrange("p (b h w) -> p b h w", b=4, w=WP)
                ha3 = hpas[g % 2].rearrange("p (h w) -> p h w", w=WP)
                hb3 = hpbs[g % 2].rearrange("p (h w) -> p h w", w=WP)
                for c in range(2):
                    r0 = 16 * c
                    psa = psp.tile([128, 512], f32, name=f"ps1a_{g}_{c}", tag="ps")
                    psb = psp.tile([128, 512], f32, name=f"ps1b_{g}_{c}", tag="ps")
                    for t in range(9):
                        dy, dx = divmod(t, 3)
                        for k in range(8):
                            row = 64 * (k // 4)
                            col = 32 * (k % 4)
                            ps = psa if k < 4 else psb
                            nc.tensor.matmul(
                                ps[col:col + 32, :],
                                lhsT=w1sb[row:row + 64,
                                          (k % 4) * 288 + t * 32:(k % 4) * 288 + (t + 1) * 32],
                                rhs=xp4[row:row + 64, k % 4,
                                        r0 + dy:r0 + dy + 16, dx:dx + HW],
                                start=(t == 0), stop=(t == 8),
                                tile_position=(row, col),
                            )
                    # evacs on scalar; the last chunk's bank-b goes to
                    # vector (idle then) so all banks free ~0.7us sooner -
                    # the next conv2's psum allocation waits on these
                    nc.scalar.activation(
                        ha3[:, r0 + 1:r0 + 17, 1:HW + 1],
                        psa.rearrange("p (h w) -> p h w", w=HW),
                        Relu, bias=b1t[:, 2 * g:2 * g + 1])
                    if c == 1:
                        nc.vector.tensor_scalar(
                            hb3[:, r0 + 1:r0 + 17, 1:HW + 1],
                            psb.rearrange("p (h w) -> p h w", w=HW),
                            b1t[:, 2 * g + 1:2 * g + 2], 0.0,
                            op0=add, op1=amax)
                    else:
                        nc.scalar.activation(
                            hb3[:, r0 + 1:r0 + 17, 1:HW + 1],
                            psb.rearrange("p (h w) -> p h w", w=HW),
                            Relu, bias=b1t[:, 2 * g + 1:2 * g + 2])

            def emit_conv2(g):
                w2sb = w_tiles[g][:, 1152:2304]
                xqall = xq_tiles[g]
                ha3 = hpas[g % 2].rearrange("p (h w) -> p h w", w=WP)
                hb3 = hpbs[g % 2].rearrange("p (h w) -> p h w", w=WP)
                outs = [ospp.tile([128, 1024], bf16, tag=f"os{r}",
                                  name=f"os{r}_{g}") for r in range(4)]
                chunks = ((0, 16), (16, 16))
                for ci, (r0, nr) in enumerate(chunks):
                    nw = nr * HW
                    pss = [psp.tile([128, nw], f32, name=f"ps2_{g}_{ci}_{r}",
                                    tag="ps")
                           for r in range(4)]
                    for t in range(9):
                        dy, dx = divmod(t, 3)
                        for k in range(8):
                            r, h = divmod(k, 2)
                            src3 = ha3 if h == 0 else hb3
                            nc.tensor.matmul(
                                pss[r][64 * h:64 * h + 64, :],
                                lhsT=w2sb[32 * r:32 * r + 32,
                                          h * 576 + t * 64:h * 576 + (t + 1) * 64],
                                rhs=src3[32 * r:32 * r + 32,
                                         r0 + dy:r0 + dy + nr, dx:dx + HW],
                                start=(t == 0), stop=(t == 8),
                                tile_position=(32 * r, 64 * h),
                            )
                    o0 = r0 * HW
                    for r in range(4):
                        nc.vector.tensor_tensor(
                            outs[r][:, o0:o0 + nw], pss[r][:, :],
                            xqall[:, r * 1024 + o0:r * 1024 + o0 + nw],
                            op=add)
                        row0 = (g * 4 + r) * 128
                        if g >= NG - 2:
                            # last two groups: store per chunk, spread over
                            # all three queues, so almost no transfer remains
                            # after the final epilogue op
                            eng = (nc.sync, nc.sync, nc.gpsimd, nc.scalar)[r]
                            eng.dma_start(
                                outd.ap()[row0:row0 + 128, o0:o0 + nw],
                                outs[r][:, o0:o0 + nw])
                        elif ci == 1:
                            eng = nc.sync if r < 2 else nc.gpsimd
                            eng.dma_start(outd.ap()[row0:row0 + 128, :],
                                          outs[r][:, :])

            # relu(g0): half-quadrant ops on vector, in DMA-landing order
            xp3g0 = xpads[0].rearrange("p (b h w) -> p b h w", b=4, w=WP)
            for k, h in ((0, 0), (2, 0), (0, 1), (2, 1),
                         (1, 0), (3, 0), (1, 1), (3, 1)):
                dst = xp3g0[:, k, 1 + 16 * h:17 + 16 * h, 1:HW + 1]
                src = xq_tiles[0][:, k * 1024 + h * 512:
                                  k * 1024 + (h + 1) * 512].rearrange(
                    "p (h w) -> p h w", w=HW)
                nc.vector.tensor_scalar(
                    dst, src, b2zt[:, k:k + 1], 0.0, op0=add, op1=amax)

            # remaining border memsets (needed from conv1(g1)/conv2(g1) on)
            for k in range(4):
                zero_borders(nc.vector, xp1q[:, k])
            zero_borders(nc.gpsimd, hpas[1])
            zero_borders(nc.gpsimd, hpbs[1])

            # ---- pipeline: pair structure ----
            # relu(g1) all vector: keeps scalar free for evac(g0)
            emit_relu(1, [(0, nc.vector), (1, nc.vector),
                          (2, nc.vector), (3, nc.vector)])
            emit_conv1(0)
            # late weight loads issued from scalar after evac(g0) (~12us):
            # keeps the head DMA window clear for xq(g0/g1)+wg(g0/g1)
            for g in (2, 3):
                for half in (0, 1):
                    load_w(g, half, nc.scalar)
            emit_relu(2, [(0, nc.vector), (1, nc.vector),
                          (2, nc.vector), (3, nc.vector)])
            emit_conv1(1)
            emit_conv2(0)
            emit_relu(3, [(0, nc.vector), (1, nc.vector),
                          (2, nc.scalar), (3, nc.scalar)])
            emit_conv2(1)
            emit_conv1(2)
            emit_conv1(3)
            emit_conv2(2)
            emit_conv2(3)

    nc.compile()
    return nc


import os as _os
if _os.environ.get("LDWOPT", "0") == "1":
    import concourse.bass_utils as _bu
    if not getattr(_bu, "_ldw_patched", False):
        _orig = _bu.run_command
        def _rc(argv, **kw):
            argv = ["--enable-ldw-opt=true" if a == "--enable-ldw-opt=false" else a
                    for a in argv]
            return _orig(argv, **kw)
        _bu.run_command = _rc
        _bu._ldw_patched = True

_NC = None


def _get_nc():
    global _NC
    if _NC is None:
        _NC = _build_bass()
    return _NC


def _host_prep(x, y_index, z, W1, b1, W2, b2):
    import ml_dtypes
    idx = np.asarray(y_index).reshape(B).astype(np.int64)
    # flipped-kernel stacks: w1t [NB, C, 9, CSM], w2t [NB, CSM, 9, C]
    w1t = np.ascontiguousarray(
        W1[:, :, :, ::-1, ::-1].transpose(0, 1, 3, 4, 2)).reshape(NB, C, 9, CSM)
    w2t = np.ascontiguousarray(
        W2[:, :, :, ::-1, ::-1].transpose(0, 1, 3, 4, 2)).reshape(NB, CSM, 9, C)
    w1s = w1t[idx]                                   # [B, 64, 9, 32] f32
    w2s = w2t[idx] * z[:, None, None, :]             # [B, 32, 9, 64] f32
    b2z = b2[idx] * z                                # [B, 64]
    b1s = b1[idx]                                    # [B, 32]

    # xq = x + b2z, dense [B, 64, 1024]
    xq = (x + b2z[:, :, None, None]).reshape(B, C, HW * HW)
    xq_span = xq.astype(ml_dtypes.bfloat16)

    w1sb = w1s.astype(ml_dtypes.float8_e4m3fn)
    w2sb = w2s.astype(ml_dtypes.float8_e4m3fn)

    in_maps = []
    for cr in range(M):
        s0 = cr * BS
        # xqg rows: (g, k) pair tile = samples (s0+8g+k | s0+8g+4+k)
        xqg = np.empty((NG * 4 * 128, 1024), ml_dtypes.bfloat16)
        wgh = np.zeros((NG * 2 * 128, 1152), ml_dtypes.float8_e4m3fn)
        b1h = np.zeros((128, 2 * NG), np.float32)
        b2zh = np.zeros((128, 4 * NG), np.float32)
        for g in range(NG):
            for k in range(4):
                sa, sb = s0 + 8 * g + k, s0 + 8 * g + 4 + k
                q0 = (g * 4 + k) * 128
                xqg[q0:q0 + 64, :] = xq_span[sa]
                xqg[q0 + 64:q0 + 128, :] = xq_span[sb]
                # relu(x) = max(xq - b2z, 0): bias is ADDED by ACT/DVE
                b2zh[0:64, 4 * g + k] = -b2z[sa]
                b2zh[64:128, 4 * g + k] = -b2z[sb]
                # conv1 weights: tile k (cols k*288) top=sa, tile 8+k bottom=sb
                w1r0 = 2 * g * 128
                wgh[w1r0:w1r0 + 64,
                    k * 288:(k + 1) * 288] = w1sb[sa].reshape(64, 288)
                wgh[w1r0 + 64:w1r0 + 128,
                    k * 288:(k + 1) * 288] = w1sb[sb].reshape(64, 288)
                # conv1 bias: bank a (cols 2g) = samples sa at 32*k..; bank b = sb
                b1h[32 * k:32 * (k + 1), 2 * g] = b1s[sa]
                b1h[32 * k:32 * (k + 1), 2 * g + 1] = b1s[sb]
                # conv2 weights: tile (32k, 64h): h=0 -> sa, h=1 -> sb
                w2r0 = (2 * g + 1) * 128
                wgh[w2r0 + 32 * k:w2r0 + 32 * (k + 1),
                    0:576] = w2sb[sa].reshape(32, 576)
                wgh[w2r0 + 32 * k:w2r0 + 32 * (k + 1),
                    576:1152] = w2sb[sb].reshape(32, 576)
        in_maps.append(dict(xqg=xqg, wg=wgh, b1g=b1h, b2zg=b2zh))
    return in_maps


def kernel(x, y_index, y_hard, z, W1, b1, W2, b2, _trace=False):
    x = np.asarray(x, dtype=np.float32)
    z = np.asarray(z, dtype=np.float32)
    W1 = np.asarray(W1, dtype=np.float32)
    b1 = np.asarray(b1, dtype=np.float32)
    W2 = np.asarray(W2, dtype=np.float32)
    b2 = np.asarray(b2, dtype=np.float32)

    nc = _get_nc()
    in_maps = _host_prep(x, y_index, z, W1, b1, W2, b2)
    res = run_bass_kernel_spmd(nc, in_maps, core_ids=list(range(M)), trace=_trace)
    out = np.empty((B, C, HW, HW), np.float32)
    for cr in range(M):
        o = np.asarray(res.results[cr]["out"], dtype=np.float32)
        o = o.reshape(NG, 4, 2, C, HW, HW)
        for g in range(NG):
            for k in range(4):
                out[cr * BS + 8 * g + k] = o[g, k, 0]
                out[cr * BS + 8 * g + 4 + k] = o[g, k, 1]
    if _trace:
        kernel._last_results = res
    return out


# revision 53
# speedup vs baseline: 1.0379x; 1.0039x over previous
"""Trainium2 Bass kernel for nn_DecSwitchedDeconv — PE-array-tiled per-sample convs.

Strategy (data-parallel, 32 samples/core, 4 groups of 8):
  - conv1 runs as 8 concurrent (64x32) PE tiles (tile_position row/col groups),
    conv2 as 8 concurrent (32x64) tiles: one sample per tile, zero wasted MACs.
    Each conv is 2 chunks of 16 output rows; the matmul rhs is a 3D window AP
    over the padded input so PSUM/outputs are dense 512-wide slabs. Sustained
    round cadence ~285ns (512-col stream + LDW/dispatch overhead; walrus emits
    one LDWEIGHTS per matmul, so the weight-load port is a co-bottleneck).
  - All routing/gather on host: per-sample weights gathered by y_index, flipped
    + transposed, z folded into W2, b2*z folded into the residual xq = x+b2*z.
    Weights are fp8e4 (halves weight DMA; error well within tolerance), xq and
    the output stay bf16. DRAM layouts are quadrant-major so every DMA chunk
    is a contiguous 128-row read (strided reads crater the DMA rate).
  - Head: loads issued in priority waves over the three DMA-capable queues
    (sync/gpsimd/scalar, ~100GB/s each): wave 1 = xq(g0) half-quadrants +
    w1(g0) + biases balanced so they finish together; then g1, then the rest.
    relu(g0) chases the landings as 8 half-quadrant vector ops; 22 full-array
    warmup matmuls keep the PE busy (and the HAM clock-gate warm) until the
    first conv data lands (~DMA-bound at ~12us).
  - Steady state: pair-structured PE emission c1(2p), c1(2p+1), c2(2p),
    c2(2p+1) halves tiling-mode switches and hides the conv1->conv2 evac
    latency. conv1 evacs (bias+relu) on scalar ACT, except each group's last
    chunk bank-b on vector so all PSUM banks free early (the next conv2's
    bank rotation waits on them). relu(g1..g3) on vector/scalar only (gpsimd
    tensor_scalar is ~30x slower on this shape - never use it). conv2
    epilogue (psum + xq) on vector only (sole engine that can tensor-tensor
    from PSUM). Stores: [128,1024] per row tile on sync/gpsimd; the last two
    groups store per 512-chunk spread over all three queues so almost no DMA
    drain remains after the final epilogue op.
"""

import numpy as np

import concourse.bacc as bacc
import concourse.bass as bass
import concourse.mybir as mybir
import concourse.tile as tile
from concourse.bass_utils import run_bass_kernel_spmd

B, C, CSM, NB, HW = 256, 64, 32, 8, 32
M = 8                    # cores
BS = B // M              # 32 samples/core
NG = BS // 8             # 4 groups of 8 samples
WP = HW + 2              # 34
L = WP * WP              # 1156

f32 = mybir.dt.float32
bf16 = mybir.dt.bfloat16
fp8 = mybir.dt.float8e4


def _build_bass():
    nc = bacc.Bacc(target_bir_lowering=False, debug=False)
    # per-core inputs (host pre-gathered/packed, all static)
    # xqg is quadrant-major [(g,k,pair),1024] so every [128,1024] chunk load
    # is a contiguous 256KB HBM read (strided reads gate the ramp otherwise)
    xqg = nc.dram_tensor("xqg", [NG * 4 * 128, 1024], bf16, kind="ExternalInput")
    # wg split into contiguous conv1/conv2 halves: block (g,0)=w1, (g,1)=w2
    wg = nc.dram_tensor("wg", [NG * 2 * 128, 1152], fp8, kind="ExternalInput")
    b1g = nc.dram_tensor("b1g", [128, 2 * NG], f32, kind="ExternalInput")
    b2zg = nc.dram_tensor("b2zg", [128, 4 * NG], f32, kind="ExternalInput")
    outd = nc.dram_tensor("out", [NG * 4 * 128, 1024], bf16, kind="ExternalOutput")

    add = mybir.AluOpType.add
    amax = mybir.AluOpType.max
    Relu = mybir.ActivationFunctionType.Relu

    with tile.TileContext(nc) as tc:
        # persistent padded tensors; borders zeroed once and never rewritten
        xpads, hpas, hpbs = [], [], []
        for i in range(2):
            xp = nc.alloc_sbuf_tensor(f"xpad{i}", [128, 4 * L], bf16).ap()
            ha = nc.alloc_sbuf_tensor(f"hpa{i}", [128, L], bf16).ap()
            hb = nc.alloc_sbuf_tensor(f"hpb{i}", [128, L], bf16).ap()
            xpads.append(xp)
            hpas.append(ha)
            hpbs.append(hb)
        warm_sb = nc.alloc_sbuf_tensor("warm_sb", [128, 1152], bf16).ap()
        b1t = nc.alloc_sbuf_tensor("b1t", [128, 2 * NG], f32).ap()
        b2zt = nc.alloc_sbuf_tensor("b2zt", [128, 4 * NG], f32).ap()

        def zero_borders(eng, t):
            # pad frame of one 34x34 block: row 0, row 33, cols 0/33
            eng.memset(t[:, 0:WP], 0.0)
            eng.memset(t[:, 33 * WP:L], 0.0)
            sides = t[:, 33:33 + 33 * WP].rearrange(
                "p (r w) -> p r w", w=WP)[:, :, 0:2]
            eng.memset(sides, 0.0)

        act_scr = nc.alloc_sbuf_tensor("act_scr", [128, 2], f32).ap()

        with (
            tc.tile_pool(name="io", bufs=4) as iop,
            tc.tile_pool(name="wp", bufs=4) as wpp,
            tc.tile_pool(name="osp", bufs=3) as ospp,
            tc.tile_pool(name="ps", bufs=8, space="PSUM") as psp,
        ):
            # ---- head: queue all input DMAs immediately ----
            xq_tiles, w_tiles = [], []
            for g in range(NG):
                xq_tiles.append(iop.tile([128, 4 * 1024], bf16, tag="xq",
                                         name=f"xq_{g}"))
                w_tiles.append(wpp.tile([128, 2304], fp8, tag="w",
                                        name=f"w_{g}"))
            # all xq loads are contiguous per-quadrant [128,1024] chunks.
            # g0 split sync/gpsimd for earliest landing; later groups queue
            # behind on sync - its ~0.7us/instr issue rate self-paces them.
            def load_xq(g, k, eng):
                r0 = (g * 4 + k) * 128
                eng.dma_start(xq_tiles[g][:, k * 1024:(k + 1) * 1024],
                              xqg.ap()[r0:r0 + 128, :])

            def load_w(g, half, eng):
                r0 = (g * 2 + half) * 128
                eng.dma_start(w_tiles[g][:, half * 1152:(half + 1) * 1152],
                              wg.ap()[r0:r0 + 128, :])

            # half-quadrant loads for g0 so relu can chase the DMA closely
            def load_xq_half(g, k, h, eng):
                r0 = (g * 4 + k) * 128
                eng.dma_start(
                    xq_tiles[g][:, k * 1024 + h * 512:k * 1024 + (h + 1) * 512],
                    xqg.ap()[r0:r0 + 128, h * 512:(h + 1) * 512])

            # wave 1 (critical for conv1(g0)): xq(g0) + w1(g0) + biases,
            # balanced across the three queues so they finish together
            load_w(0, 0, nc.gpsimd)                 # w1(g0), 148KB
            load_xq_half(0, 0, 0, nc.sync)
            nc.scalar.dma_start(b2zt, b2zg.ap())
            nc.scalar.dma_start(b1t, b1g.ap())
            load_xq_half(0, 2, 0, nc.scalar)
            load_xq_half(0, 0, 1, nc.sync)
            load_xq_half(0, 2, 1, nc.scalar)
            load_xq_half(0, 3, 0, nc.gpsimd)
            load_xq_half(0, 1, 0, nc.sync)
            load_xq_half(0, 1, 1, nc.scalar)
            load_xq_half(0, 3, 1, nc.gpsimd)
            # wave 2: g1's inputs
            load_xq(1, 0, nc.sync)
            load_xq(1, 2, nc.gpsimd)
            load_w(1, 0, nc.scalar)                 # w1(g1)
            load_xq(1, 1, nc.sync)
            load_xq(1, 3, nc.gpsimd)
            # wave 3: conv2 weights + later groups
            load_w(0, 1, nc.gpsimd)                 # w2(g0)
            load_w(1, 1, nc.scalar)                 # w2(g1)
            for g in range(2, NG):
                for k in range(4):
                    load_xq(g, k, nc.sync)

            # border memsets: vector does xpad0, gpsimd does hpa0/hpb0
            xp0q = xpads[0].rearrange("p (b l) -> p b l", b=4)
            xp1q = xpads[1].rearrange("p (b l) -> p b l", b=4)
            for k in range(4):
                zero_borders(nc.vector, xp0q[:, k])
            zero_borders(nc.gpsimd, hpas[0])
            zero_borders(nc.gpsimd, hpbs[0])

            # ACT table warm on a private scratch (no deps with real tiles)
            nc.scalar.activation(act_scr[:, 1:2], act_scr[:, 0:1], Relu)

            # HAM pre-warm: dummy full-array matmuls while first loads land
            wps = psp.tile([128, 512], f32, name="warm_ps", tag="ps")
            for i in range(28):
                nc.tensor.matmul(
                    wps[:, :], lhsT=warm_sb[:, 1024:1152],
                    rhs=warm_sb[:, 0:512], start=True, stop=True)

            def emit_relu(g, quads):
                # quads: list of (k, engine) quadrant assignments.
                # vector: tensor_scalar add+max; scalar: ACT relu w/ bias.
                # (gpsimd tensor_scalar is ~30x slower on this shape - avoid.)
                xqall = xq_tiles[g]
                xp3 = xpads[g % 2].rearrange("p (b h w) -> p b h w", b=4, w=WP)
                for k, eng in quads:
                    dst = xp3[:, k, 1:HW + 1, 1:HW + 1]
                    src = xqall[:, k * 1024:(k + 1) * 1024].rearrange(
                        "p (h w) -> p h w", w=HW)
                    if eng is nc.scalar:
                        eng.activation(dst, src, Relu,
                                       bias=b2zt[:, 4 * g + k:4 * g + k + 1])
                    else:
                        eng.tensor_scalar(
                            dst, src, b2zt[:, 4 * g + k:4 * g + k + 1], 0.0,
                            op0=add, op1=amax)

            def emit_conv1(g):
                w1sb = w_tiles[g][:, 0:1152]
                xp4 = xpads[g % 2].rearrange("p (b h w) -> p b h w", b=4, w=WP)
                ha3 = hpas[g % 2].rearrange("p (h w) -> p h w", w=WP)
                hb3 = hpbs[g % 2].rearrange("p (h w) -> p h w", w=WP)
                for c in range(2):
                    r0 = 16 * c
                    psa = psp.tile([128, 512], f32, name=f"ps1a_{g}_{c}", tag="ps")
                    psb = psp.tile([128, 512], f32, name=f"ps1b_{g}_{c}", tag="ps")
                    for t in range(9):
                        dy, dx = divmod(t, 3)
                        for k in range(8):
                            row = 64 * (k // 4)
                            col = 32 * (k % 4)
                            ps = psa if k < 4 else psb
                            nc.tensor.matmul(
                                ps[col:col + 32, :],
                                lhsT=w1sb[row:row + 64,
                                          (k % 4) * 288 + t * 32:(k % 4) * 288 + (t + 1) * 32],
                                rhs=xp4[row:row + 64, k % 4,
                                        r0 + dy:r0 + dy + 16, dx:dx + HW],
                                start=(t == 0), stop=(t == 8),
                                tile_position=(row, col),
                            )
                    # evacs on scalar; the last chunk's bank-b goes to
                    # vector (idle then) so all banks free ~0.7us sooner -
                    # the next conv2's psum allocation waits on these
                    nc.scalar.activation(
                        ha3[:, r0 + 1:r0 + 17, 1:HW + 1],
                        psa.rearrange("p (h w) -> p h w", w=HW),
                        Relu, bias=b1t[:, 2 * g:2 * g + 1])
                    if c == 1:
                        nc.vector.tensor_scalar(
                            hb3[:, r0 + 1:r0 + 17, 1:HW + 1],
                            psb.rearrange("p (h w) -> p h w", w=HW),
                            b1t[:, 2 * g + 1:2 * g + 2], 0.0,
                            op0=add, op1=amax)
                    else:
                        nc.scalar.activation(
                            hb3[:, r0 + 1:r0 + 17, 1:HW + 1],
                            psb.rearrange("p (h w) -> p h w", w=HW),
                            Relu, bias=b1t[:, 2 * g + 1:2 * g + 2])

            def emit_conv2(g):
                w2sb = w_tiles[g][:, 1152:2304]
                xqall = xq_tiles[g]
                ha3 = hpas[g % 2].rearrange("p (h w) -> p h w", w=WP)
                hb3 = hpbs[g % 2].rearrange("p (h w) -> p h w", w=WP)
                outs = [ospp.tile([128, 1024], bf16, tag=f"os{r}",
                                  name=f"os{r}_{g}") for r in range(4)]
                chunks = ((0, 16), (16, 16))
                for ci, (r0, nr) in enumerate(chunks):
                    nw = nr * HW
                    pss = [psp.tile([128, nw], f32, name=f"ps2_{g}_{ci}_{r}",
                                    tag="ps")
                           for r in range(4)]
                    for t in range(9):
                        dy, dx = divmod(t, 3)
                        for k in range(8):
                            r, h = divmod(k, 2)
                            src3 = ha3 if h == 0 else hb3
                            nc.tensor.matmul(
                                pss[r][64 * h:64 * h + 64, :],
                                lhsT=w2sb[32 * r:32 * r + 32,
                                          h * 576 + t * 64:h * 576 + (t + 1) * 64],
                                rhs=src3[32 * r:32 * r + 32,
                                         r0 + dy:r0 + dy + nr, dx:dx + HW],
                                start=(t == 0), stop=(t == 8),
                                tile_position=(32 * r, 64 * h),
                            )
                    o0 = r0 * HW
                    for r in range(4):
                        nc.vector.tensor_tensor(
                            outs[r][:, o0:o0 + nw], pss[r][:, :],
                            xqall[:, r * 1024 + o0:r * 1024 + o0 + nw],
                            op=add)
                        row0 = (g * 4 + r) * 128
                        if g >= NG - 2:
                            # last two groups: store per chunk, spread over
                            # all three queues, so almost no transfer remains
                            # after the final epilogue op
                            eng = (nc.sync, nc.sync, nc.gpsimd, nc.scalar)[r]
                            eng.dma_start(
                                outd.ap()[row0:row0 + 128, o0:o0 + nw],
                                outs[r][:, o0:o0 + nw])
                        elif ci == 1:
                            eng = nc.sync if r < 2 else nc.gpsimd
                            eng.dma_start(outd.ap()[row0:row0 + 128, :],
                                          outs[r][:, :])

            # relu(g0): half-quadrant ops on vector, in DMA-landing order
            xp3g0 = xpads[0].rearrange("p (b h w) -> p b h w", b=4, w=WP)
            for k, h in ((0, 0), (2, 0), (0, 1), (2, 1),
                         (1, 0), (3, 0), (1, 1), (3, 1)):
                dst = xp3g0[:, k, 1 + 16 * h:17 + 16 * h, 1:HW + 1]
                src = xq_tiles[0][:, k * 1024 + h * 512:
                                  k * 1024 + (h + 1) * 512].rearrange(
                    "p (h w) -> p h w", w=HW)
                nc.vector.tensor_scalar(
                    dst, src, b2zt[:, k:k + 1], 0.0, op0=add, op1=amax)

            # remaining border memsets (needed from conv1(g1)/conv2(g1) on)
            for k in range(4):
                zero_borders(nc.vector, xp1q[:, k])
            zero_borders(nc.gpsimd, hpas[1])
            zero_borders(nc.gpsimd, hpbs[1])

            # ---- pipeline: pair structure ----
            # relu(g1) all vector: keeps scalar free for evac(g0)
            emit_relu(1, [(0, nc.vector), (1, nc.vector),
                          (2, nc.vector), (3, nc.vector)])
            emit_conv1(0)
            # late weight loads issued from scalar after evac(g0) (~12us):
            # keeps the head DMA window clear for xq(g0/g1)+wg(g0/g1)
            for g in (2, 3):
                for half in (0, 1):
                    load_w(g, half, nc.scalar)
            emit_relu(2, [(0, nc.vector), (1, nc.vector),
                          (2, nc.vector), (3, nc.vector)])
            emit_conv1(1)
            emit_conv2(0)
            emit_relu(3, [(0, nc.vector), (1, nc.vector),
                          (2, nc.scalar), (3, nc.scalar)])
            emit_conv2(1)
            emit_conv1(2)
            emit_conv1(3)
            emit_conv2(2)
            emit_conv2(3)

    nc.compile()
    return nc


import os as _os
if _os.environ.get("LDWOPT", "0") == "1":
    import concourse.bass_utils as _bu
    if not getattr(_bu, "_ldw_patched", False):
        _orig = _bu.run_command
        def _rc(argv, **kw):
            argv = ["--enable-ldw-opt=true" if a == "--enable-ldw-opt=false" else a
                    for a in argv]
            return _orig(argv, **kw)
        _bu.run_command = _rc
        _bu._ldw_patched = True

_NC = None


def _get_nc():
    global _NC
    if _NC is None:
        _NC = _build_bass()
    return _NC


def _host_prep(x, y_index, z, W1, b1, W2, b2):
    import ml_dtypes
    idx = np.asarray(y_index).reshape(B).astype(np.int64)
    # flipped-kernel stacks: w1t [NB, C, 9, CSM], w2t [NB, CSM, 9, C]
    w1t = np.ascontiguousarray(
        W1[:, :, :, ::-1, ::-1].transpose(0, 1, 3, 4, 2)).reshape(NB, C, 9, CSM)
    w2t = np.ascontiguousarray(
        W2[:, :, :, ::-1, ::-1].transpose(0, 1, 3, 4, 2)).reshape(NB, CSM, 9, C)
    w1s = w1t[idx]                                   # [B, 64, 9, 32] f32
    w2s = w2t[idx] * z[:, None, None, :]             # [B, 32, 9, 64] f32
    b2z = b2[idx] * z                                # [B, 64]
    b1s = b1[idx]                                    # [B, 32]

    # xq = x + b2z, dense [B, 64, 1024]
    xq = (x + b2z[:, :, None, None]).reshape(B, C, HW * HW)
    xq_span = xq.astype(ml_dtypes.bfloat16)

    w1sb = w1s.astype(ml_dtypes.float8_e4m3fn)
    w2sb = w2s.astype(ml_dtypes.float8_e4m3fn)

    in_maps = []
    for cr in range(M):
        s0 = cr * BS
        # xqg rows: (g, k) pair tile = samples (s0+8g+k | s0+8g+4+k)
        xqg = np.empty((NG * 4 * 128, 1024), ml_dtypes.bfloat16)
        wgh = np.zeros((NG * 2 * 128, 1152), ml_dtypes.float8_e4m3fn)
        b1h = np.zeros((128, 2 * NG), np.float32)
        b2zh = np.zeros((128, 4 * NG), np.float32)
        for g in range(NG):
            for k in range(4):
                sa, sb = s0 + 8 * g + k, s0 + 8 * g + 4 + k
                q0 = (g * 4 + k) * 128
                xqg[q0:q0 + 64, :] = xq_span[sa]
                xqg[q0 + 64:q0 + 128, :] = xq_span[sb]
                # relu(x) = max(xq - b2z, 0): bias is ADDED by ACT/DVE
                b2zh[0:64, 4 * g + k] = -b2z[sa]
                b2zh[64:128, 4 * g + k] = -b2z[sb]
                # conv1 weights: tile k (cols k*288) top=sa, tile 8+k bottom=sb
                w1r0 = 2 * g * 128
                wgh[w1r0:w1r0 + 64,
                    k * 288:(k + 1) * 288] = w1sb[sa].reshape(64, 288)
                wgh[w1r0 + 64:w1r0 + 128,
                    k * 288:(k + 1) * 288] = w1sb[sb].reshape(64, 288)
                # conv1 bias: bank a (cols 2g) = samples sa at 32*k..; bank b = sb
                b1h[32 * k:32 * (k + 1), 2 * g] = b1s[sa]
                b1h[32 * k:32 * (k + 1), 2 * g + 1] = b1s[sb]
                # conv2 weights: tile (32k, 64h): h=0 -> sa, h=1 -> sb
                w2r0 = (2 * g + 1) * 128
                wgh[w2r0 + 32 * k:w2r0 + 32 * (k + 1),
                    0:576] = w2sb[sa].reshape(32, 576)
                wgh[w2r0 + 32 * k:w2r0 + 32 * (k + 1),
                    576:1152] = w2sb[sb].reshape(32, 576)
        in_maps.append(dict(xqg=xqg, wg=wgh, b1g=b1h, b2zg=b2zh))
    return in_maps


def kernel(x, y_index, y_hard, z, W1, b1, W2, b2, _trace=False):
    x = np.asarray(x, dtype=np.float32)
    z = np.asarray(z, dtype=np.float32)
    W1 = np.asarray(W1, dtype=np.float32)
    b1 = np.asarray(b1, dtype=np.float32)
    W2 = np.asarray(W2, dtype=np.float32)
    b2 = np.asarray(b2, dtype=np.float32)

    nc = _get_nc()
    in_maps = _host_prep(x, y_index, z, W1, b1, W2, b2)
    res = run_bass_kernel_spmd(nc, in_maps, core_ids=list(range(M)), trace=_trace)
    out = np.empty((B, C, HW, HW), np.float32)
    for cr in range(M):
        o = np.asarray(res.results[cr]["out"], dtype=np.float32)
        o = o.reshape(NG, 4, 2, C, HW, HW)
        for g in range(NG):
            for k in range(4):
                out[cr * BS + 8 * g + k] = o[g, k, 0]
                out[cr * BS + 8 * g + 4 + k] = o[g, k, 1]
    if _trace:
        kernel._last_results = res
    return out
